# revision 1
# baseline (speedup 1.0000x reference)
"""Two-layer GATConv (PyG-style, edge_dim=1, add_self_loops fill='mean') on
8 trn2 NeuronCores.

Strategy
--------
Destinations are partitioned across the 8 cores (degree-sorted, dealt
round-robin so every core sees the same per-tile max degree).  Each
destination-tile of 128 dsts is processed with a fixed slot grid
[128 dsts x K_t slots]; slot K_t-1 is the self-loop, the rest are the
in-edges (padded).  The host pre-gathers the *input features of each
slot's source node* into a dense [128, SLOTS] operand (pure data
movement - every FLOP runs on device), so the device pipeline is:

  per chunk k:  psum = x_slot_chunk^T.T @ Wcat   (PE; Wcat = [W | W@A_src | W@A_dst])
  wide DVE:     alpha = a_src + a_dst + w*k_h ; self-loop alpha via
                loop_attr = (sum w) * 1/max(cnt,1) ; leaky-relu ; exp (ACT)
  accumulate:   O += p_kh * h_chunk  (DVE scalar_tensor_tensor, per chunk/head)
  epilogue:     out = O / Z + b  (+ relu for layer 1)

Layer 2 re-uses the identical slot structure; its slot features are the
layer-1 output rows (host-gathered between the two device programs).
Padding slots use a host-computed x-direction whose a_src projection is
-1e4 for every head, so exp(alpha_pad) == 0 exactly.
"""
import copy
import os

import numpy as np

import concourse.bass as bass
import concourse.mybir as mybir
import concourse.tile as tile
from contextlib import ExitStack
from concourse.bass_utils import run_bass_kernel_spmd

NCORES = 8
P = 128
N = 50000
E = 800000
IN_DIM = 128
NPC = N // NCORES            # 6250 dsts per core
T = (NPC + P - 1) // P       # 49 tiles
NROWS = T * P                # 6272 rows per core (incl pad dsts)
NEG_SLOPE = 0.2

F32 = mybir.dt.float32

# exec-time bookkeeping for test.py (populated when BASSGNN_TRACE=1)
LAST_EXEC_NS = []


# --------------------------------------------------------------------------
# walrus workaround: this container's walrus caps sync waits per instruction
# at ~2 (and adds its own to Drain/Branch).  Move excess waits onto
# InstEventSemaphore carriers emitted just before the over-limit instruction.
# --------------------------------------------------------------------------
def _split_waits(nc, limit=1):
    sem = nc.alloc_semaphore("wsplit_tmpl_sem")
    tmpl = {}
    for eng_ty, eng in nc.engines.items():
        tmpl[eng_ty] = eng.wait_ge(sem, 0).ins
    tmpl_names = {mi.name for mi in tmpl.values()}
    for f in nc.m.functions:
        for bb in f.blocks:
            insts = [i for i in bb.instructions if i.name not in tmpl_names]
            out = []
            for inst in insts:
                si = inst.sync_info
                waits = list(si.on_wait) if si and si.on_wait else []
                tn = type(inst).__name__
                eff = 0 if (tn == "InstDrain" or "Branch" in tn) else limit
                if len(waits) > eff:
                    head = waits[:-eff] if eff else waits
                    for w in head:
                        c = copy.deepcopy(tmpl[inst.engine])
                        c.name = f"I-wsplit-{nc.next_id()}"
                        c.sync_info = mybir.SyncInfo(on_wait=[w], on_update=[])
                        out.append(c)
                    inst.sync_info = mybir.SyncInfo(
                        on_wait=waits[-eff:] if eff else [],
                        on_update=list(si.on_update) if si.on_update else [],
                    )
                out.append(inst)
            bb.instructions = out


def _ap(root, extra_off, dims):
    """Build an AP on root's tensor at root.offset+extra_off with explicit
    [step, count] dims."""
    return bass.AP(root.tensor, root.offset + extra_off, [list(d) for d in dims])


# --------------------------------------------------------------------------
# device program: one GAT layer over the slot grid
# --------------------------------------------------------------------------
def _build_layer(KT, H, C, relu):
    """KT: per-tile slot counts. Input feature dim is always 128.
    Wcat columns: [h (H*C) | a_src (H) | a_dst (H)]  -> COLS = H*C + 2*H."""
    HC = H * C
    COLS = HC + 2 * H
    SK = sum(KT)                 # total chunks
    SLOTS = P * SK

    nc = bass.Bass()
    xsT = nc.dram_tensor("xsT", [P, SLOTS], F32, kind="ExternalInput")
    wcat = nc.dram_tensor("wcat", [P, COLS], F32, kind="ExternalInput")
    kk = nc.dram_tensor("kk", [P, H], F32, kind="ExternalInput")
    warr = nc.dram_tensor("warr", [P, SK], F32, kind="ExternalInput")
    invc = nc.dram_tensor("invc", [P, T], F32, kind="ExternalInput")
    bvec = nc.dram_tensor("bvec", [P, HC], F32, kind="ExternalInput")
    outp = nc.dram_tensor("out", [NROWS, HC], F32, kind="ExternalOutput")

    with ExitStack() as ctx:
        tc = ctx.enter_context(tile.TileContext(nc))
        pers = ctx.enter_context(tc.tile_pool(name="pers", bufs=1))
        xp = ctx.enter_context(tc.tile_pool(name="xp", bufs=2))
        gp = ctx.enter_context(tc.tile_pool(name="gp", bufs=2))
        sb = ctx.enter_context(tc.tile_pool(name="sb", bufs=3))
        ps = ctx.enter_context(tc.tile_pool(name="ps", bufs=2, space="PSUM"))

        wc = pers.tile([P, COLS], F32)
        nc.sync.dma_start(out=wc[:], in_=wcat[:, :])
        kt = pers.tile([P, H], F32)
        nc.sync.dma_start(out=kt[:], in_=kk[:, :])
        bt = pers.tile([P, HC], F32)
        nc.sync.dma_start(out=bt[:], in_=bvec[:, :])
        wall = pers.tile([P, SK], F32)
        nc.sync.dma_start(out=wall[:], in_=warr[:, :])
        iva = pers.tile([P, T], F32)
        nc.sync.dma_start(out=iva[:], in_=invc[:, :])

        cb = 0  # chunk base
        for t in range(T):
            K = KT[t]
            xt = xp.tile([P, K * P], F32, tag="xt")
            nc.sync.dma_start(out=xt[:], in_=xsT[:, cb * P:(cb + K) * P])
            G = gp.tile([P, K * COLS], F32, tag="G")
            for k in range(K):
                pg = ps.tile([P, COLS], F32, tag="pg")
                nc.tensor.matmul(out=pg[:], lhsT=xt[:, k * P:(k + 1) * P],
                                 rhs=wc[:], start=True, stop=True)
                nc.scalar.copy(out=G[:, k * COLS:(k + 1) * COLS], in_=pg[:])

            g0 = G[:]
            gpitch = g0.ap[0][0]
            # alpha = a_src[slot] + a_dst[dst] + w*k_h   ([P, K, H])
            A = sb.tile([P, K * H], F32, tag="A")
            a0 = A[:]
            apitch = a0.ap[0][0]
            A3 = _ap(a0, 0, [(apitch, P), (H, K), (1, H)])
            asrc = _ap(g0, HC, [(gpitch, P), (COLS, K), (1, H)])
            adstb = _ap(g0, (K - 1) * COLS + HC + H, [(gpitch, P), (0, K), (1, H)])
            nc.vector.tensor_tensor(out=A3, in0=asrc, in1=adstb,
                                    op=mybir.AluOpType.add)
            wt0 = wall[:, cb:cb + K]
            wpitch = wt0.ap[0][0]
            WK = sb.tile([P, K * H], F32, tag="WK")
            wk0 = WK[:]
            WK3 = _ap(wk0, 0, [(wk0.ap[0][0], P), (H, K), (1, H)])
            w_b = _ap(wt0, 0, [(wpitch, P), (1, K), (0, H)])
            kk_b = _ap(kt[:], 0, [(kt[:].ap[0][0], P), (0, K), (1, H)])
            nc.vector.tensor_tensor(out=WK3, in0=w_b, in1=kk_b,
                                    op=mybir.AluOpType.mult)
            nc.vector.tensor_tensor(out=A3, in0=A3, in1=WK3,
                                    op=mybir.AluOpType.add)
            # self-loop alpha at slot K-1
            LA = sb.tile([P, 1], F32, tag="LA")
            nc.vector.tensor_reduce(out=LA[:], in_=wt0,
                                    axis=mybir.AxisListType.X,
                                    op=mybir.AluOpType.add)
            nc.vector.tensor_tensor(out=LA[:], in0=LA[:], in1=iva[:, t:t + 1],
                                    op=mybir.AluOpType.mult)
            TSD = sb.tile([P, H], F32, tag="TSD")
            self_asrc = _ap(g0, (K - 1) * COLS + HC, [(gpitch, P), (1, H)])
            self_adst = _ap(g0, (K - 1) * COLS + HC + H, [(gpitch, P), (1, H)])
            nc.vector.tensor_tensor(out=TSD[:], in0=self_asrc, in1=self_adst,
                                    op=mybir.AluOpType.add)
            A_self = _ap(a0, (K - 1) * H, [(apitch, P), (1, H)])
            kk_b2 = _ap(kt[:], 0, [(kt[:].ap[0][0], P), (1, H)])
            nc.vector.scalar_tensor_tensor(out=A_self, in0=kk_b2, scalar=LA[:],
                                           in1=TSD[:],
                                           op0=mybir.AluOpType.mult,
                                           op1=mybir.AluOpType.add)
            # leaky relu (DVE) + exp (ACT)
            AL = sb.tile([P, K * H], F32, tag="AL")
            nc.vector.tensor_scalar_mul(out=AL[:], in0=A[:], scalar1=NEG_SLOPE)
            nc.vector.tensor_tensor(out=AL[:], in0=AL[:], in1=A[:],
                                    op=mybir.AluOpType.max)
            PP = sb.tile([P, K * H], F32, tag="PP")
            nc.scalar.activation(out=PP[:], in_=AL[:],
                                 func=mybir.ActivationFunctionType.Exp)
            # O = sum_k p_kh * h_k
            O = sb.tile([P, HC], F32, tag="O")
            nc.vector.memset(O[:], 0.0)
            for k in range(K):
                for h in range(H):
                    hk = _ap(g0, k * COLS + h * C, [(gpitch, P), (1, C)])
                    nc.vector.scalar_tensor_tensor(
                        out=O[:, h * C:(h + 1) * C], in0=hk,
                        scalar=PP[:, k * H + h:k * H + h + 1],
                        in1=O[:, h * C:(h + 1) * C],
                        op0=mybir.AluOpType.mult, op1=mybir.AluOpType.add)
            # Z, 1/Z, epilogue
            Z = sb.tile([P, H], F32, tag="Z")
            pp0 = PP[:]
            PPr = _ap(pp0, 0, [(pp0.ap[0][0], P), (1, H), (H, K)])
            nc.vector.tensor_reduce(out=Z[:], in_=PPr,
                                    axis=mybir.AxisListType.X,
                                    op=mybir.AluOpType.add)
            Zr = sb.tile([P, H], F32, tag="Zr")
            nc.vector.reciprocal(out=Zr[:], in_=Z[:])
            zr0 = Zr[:]
            Zrb = _ap(zr0, 0, [(zr0.ap[0][0], P), (1, H), (0, C)])
            O3 = _ap(O[:], 0, [(O[:].ap[0][0], P), (C, H), (1, C)])
            nc.vector.tensor_tensor(out=O3, in0=O3, in1=Zrb,
                                    op=mybir.AluOpType.mult)
            R = sb.tile([P, HC], F32, tag="R")
            bt_b = _ap(bt[:], 0, [(bt[:].ap[0][0], P), (1, HC)])
            nc.vector.tensor_tensor(out=R[:], in0=O[:], in1=bt_b,
                                    op=mybir.AluOpType.add)
            if relu:
                nc.vector.tensor_scalar_max(out=R[:], in0=R[:], scalar1=0.0)
            nc.sync.dma_start(out=outp[t * P:(t + 1) * P, :], in_=R[:])
            cb += K

    _split_waits(nc)
    return nc


# --------------------------------------------------------------------------
# host-side planning
# --------------------------------------------------------------------------
def _plan(edge_index):
    src = np.asarray(edge_index[0], dtype=np.int64)
    dst = np.asarray(edge_index[1], dtype=np.int64)
    deg = np.bincount(dst, minlength=N)
    order = np.argsort(-deg, kind="stable")          # rank -> node
    rank_of = np.empty(N, np.int64)
    rank_of[order] = np.arange(N)
    core_of = (rank_of % NCORES).astype(np.int64)
    loc_of = (rank_of // NCORES).astype(np.int64)

    KT = []
    for t in range(T):
        r0 = min(1024 * t, N - 1)
        KT.append(int(deg[order[r0]]) + 1)
    KT = [max(k, 1) for k in KT]
    cbs = np.concatenate([[0], np.cumsum(KT)])       # chunk bases

    # per-edge slot position: k = occurrence index within its dst
    eorder = np.argsort(dst, kind="stable")
    starts = np.concatenate([[0], np.cumsum(deg)])
    kpos_sorted = np.arange(E) - starts[dst[eorder]]
    kpos = np.empty(E, np.int64)
    kpos[eorder] = kpos_sorted

    e_core = core_of[dst]
    e_loc = loc_of[dst]
    e_t = e_loc >> 7
    e_p = e_loc & 127
    e_col = (cbs[e_t] + kpos) * P + e_p              # xsT column
    e_wcol = cbs[e_t] * 1 + kpos                     # warr column (row = e_p)

    return dict(src=src, dst=dst, deg=deg, order=order, core_of=core_of,
                loc_of=loc_of, KT=KT, cbs=cbs, kpos=kpos, e_core=e_core,
                e_t=e_t, e_p=e_p, e_col=e_col, e_wcol=e_wcol)


def _pad_dir(Wa):
    """x-direction whose projection through Wa (128 x H) is -1e4 per head."""
    Hh = Wa.shape[1]
    v, *_ = np.linalg.lstsq(Wa.T.astype(np.float64),
                            np.full(Hh, -1e4, np.float64), rcond=None)
    return v.astype(np.float32)


def _build_inputs(plan, feats, ew, Wcat, Wa_src, kvec, bias, H):
    """Per-core input maps for one layer. feats: [N, 128] source features."""
    KT, cbs = plan["KT"], plan["cbs"]
    SK = int(cbs[-1])
    SLOTS = P * SK
    vpad = _pad_dir(Wa_src)
    featsT = np.ascontiguousarray(feats.T)           # [128, N]

    src, e_core = plan["src"], plan["e_core"]
    e_col, e_wcol, e_p = plan["e_col"], plan["e_wcol"], plan["e_p"]
    order, core_of, loc_of, deg = plan["order"], plan["core_of"], plan["loc_of"], plan["deg"]

    maps = []
    for c in range(NCORES):
        xsT = np.empty((P, SLOTS), np.float32)
        xsT[:] = vpad[:, None]
        m = e_core == c
        xsT[:, e_col[m]] = featsT[:, src[m]]
        warr = np.zeros((P, SK), np.float32)
        warr[e_p[m], e_wcol[m]] = ew[m]
        # self slots: dst node's own features at chunk cbs[t]+K_t-1
        nodes = order[c::NCORES]                     # local order, len 6250
        loc = np.arange(nodes.size)
        tt = loc >> 7
        pp = loc & 127
        self_col = (cbs[tt] + np.array(KT)[tt] - 1) * P + pp
        xsT[:, self_col] = featsT[:, nodes]
        iv = np.ones((P, T), np.float32)
        iv[pp, tt] = 1.0 / np.maximum(deg[nodes], 1.0)
        maps.append({
            "xsT": xsT,
            "wcat": np.ascontiguousarray(Wcat),
            "kk": np.tile(kvec.reshape(1, H).astype(np.float32), (P, 1)),
            "warr": warr,
            "invc": iv,
            "bvec": np.tile(bias.reshape(1, -1).astype(np.float32), (P, 1)),
        })
    return maps


def _collect(plan, results, HC):
    """[8 x [NROWS, HC]] device outputs -> [N, HC] in node order."""
    stack = np.stack([r["out"] for r in results])    # [8, NROWS, HC]
    return stack[plan["core_of"], plan["loc_of"], :]


def _wcat(W, att_src, att_dst, H, C):
    """[128, H*C + 2H]: [W | W@A_src | W@A_dst]."""
    Wa_s = np.stack([W[:, h * C:(h + 1) * C] @ att_src[h] for h in range(H)], 1)
    Wa_d = np.stack([W[:, h * C:(h + 1) * C] @ att_dst[h] for h in range(H)], 1)
    return np.concatenate([W, Wa_s, Wa_d], axis=1).astype(np.float32), Wa_s


def kernel(x, edge_index, edge_weight, W1, att_src1, att_dst1, W_edge1,
           att_edge1, b1, W2, att_src2, att_dst2, W_edge2, att_edge2, b2):
    global LAST_EXEC_NS
    LAST_EXEC_NS = []
    trace = os.environ.get("BASSGNN_TRACE", "0") == "1"

    x = np.asarray(x, np.float32)
    ew = np.asarray(edge_weight, np.float32).reshape(-1)
    plan = _plan(np.asarray(edge_index))

    # k_h = W_edge[0, h*C:(h+1)*C] @ att_edge[h]  (edge_dim == 1)
    k1 = np.array([W_edge1[0, h * 64:(h + 1) * 64] @ att_edge1[h]
                   for h in range(2)], np.float32)
    k2 = np.array([W_edge2[0, :64] @ att_edge2[0]], np.float32)
    W1c, Wa_s1 = _wcat(np.asarray(W1, np.float32), np.asarray(att_src1),
                       np.asarray(att_dst1), 2, 64)
    W2c, Wa_s2 = _wcat(np.asarray(W2, np.float32), np.asarray(att_src2),
                       np.asarray(att_dst2), 1, 64)

    core_ids = list(range(NCORES))

    nc1 = _build_layer(plan["KT"], H=2, C=64, relu=True)
    maps1 = _build_inputs(plan, x, ew, W1c, Wa_s1, k1, np.asarray(b1), 2)
    r1 = run_bass_kernel_spmd(nc1, maps1, core_ids, trace=trace)
    if trace:
        LAST_EXEC_NS.append(r1.exec_time_ns)
    relu1 = _collect(plan, r1.results, 128)          # [N, 128]

    nc2 = _build_layer(plan["KT"], H=1, C=64, relu=False)
    maps2 = _build_inputs(plan, relu1, ew, W2c, Wa_s2, k2, np.asarray(b2), 1)
    r2 = run_bass_kernel_spmd(nc2, maps2, core_ids, trace=trace)
    if trace:
        LAST_EXEC_NS.append(r2.exec_time_ns)
    return _collect(plan, r2.results, 64).astype(np.float32)



# revision 4
# speedup vs baseline: 1.3981x; 1.3981x over previous
"""Two-layer GATConv (PyG-style, edge_dim=1, add_self_loops fill='mean') on
8 trn2 NeuronCores.

Strategy (v2: project-once, gather-h)
-------------------------------------
Destinations are partitioned across the 8 cores (degree-sorted, dealt
round-robin).  Three device programs per kernel call:

  P1  per-node projection: h1 = x @ [W1 | W1@A_src1 | W1@A_dst1]
      (bf16 matmul, 49 tiles/core).  Outputs h1 in bf16 and the
      per-node attention scalars in f32.
  P2  layer-1 edge aggregation over a [128 dst x K_t slot] grid whose
      slot payloads are HOST-GATHERED h1 rows (bf16) -- no per-slot
      matmul, no PSUM evacuation.  alpha = a_src + a_dst + w*k (DVE),
      exp on ACT, slot-product on Pool (GpSimd), segment-reduce on DVE,
      epilogue + relu, then the layer-2 projection of the tile's own
      output rows fused on PE (transpose + matmul).  Outputs h2 rows.
  P3  layer-2 edge aggregation, same grid, slots gathered from h2.

All FLOPs run on device; the host only moves data (gather/scatter of
rows, dtype rounding).  Slot payloads travel as bf16; accumulations
happen in f32 (PSUM / DVE reduce outputs).
"""
import copy
import os

import numpy as np
import ml_dtypes

import concourse.bass as bass
import concourse.mybir as mybir
import concourse.tile as tile
from contextlib import ExitStack
from concourse.bass_utils import run_bass_kernel_spmd

NCORES = 8
P = 128
N = 50000
E = 800000
IN_DIM = 128
NPC = N // NCORES            # 6250 dsts per core
T = (NPC + P - 1) // P       # 49 tiles
NROWS = T * P                # 6272 rows per core (incl pad dsts)
NEG_SLOPE = 0.2

F32 = mybir.dt.float32
BF16 = mybir.dt.bfloat16
NPBF = ml_dtypes.bfloat16

# exec-time bookkeeping for test.py (populated when BASSGNN_TRACE=1)
LAST_EXEC_NS = []


# --------------------------------------------------------------------------
# walrus workaround: this container's walrus caps sync waits per instruction
# at ~2 (and adds its own to Drain/Branch).  Move excess waits onto
# InstEventSemaphore carriers emitted just before the over-limit instruction.
# --------------------------------------------------------------------------
def _split_waits(nc, limit=1):
    sem = nc.alloc_semaphore("wsplit_tmpl_sem")
    tmpl = {}
    for eng_ty, eng in nc.engines.items():
        tmpl[eng_ty] = eng.wait_ge(sem, 0).ins
    tmpl_names = {mi.name for mi in tmpl.values()}
    for f in nc.m.functions:
        for bb in f.blocks:
            insts = [i for i in bb.instructions if i.name not in tmpl_names]
            out = []
            for inst in insts:
                si = inst.sync_info
                waits = list(si.on_wait) if si and si.on_wait else []
                tn = type(inst).__name__
                eff = 0 if (tn == "InstDrain" or "Branch" in tn) else limit
                if len(waits) > eff:
                    head = waits[:-eff] if eff else waits
                    for w in head:
                        c = copy.deepcopy(tmpl[inst.engine])
                        c.name = f"I-wsplit-{nc.next_id()}"
                        c.sync_info = mybir.SyncInfo(on_wait=[w], on_update=[])
                        out.append(c)
                    inst.sync_info = mybir.SyncInfo(
                        on_wait=waits[-eff:] if eff else [],
                        on_update=list(si.on_update) if si.on_update else [],
                    )
                out.append(inst)
            bb.instructions = out


def _ap(root, extra_off, dims):
    """AP on root's tensor at root.offset+extra_off with explicit
    [step, count] dims."""
    return bass.AP(root.tensor, root.offset + extra_off, [list(d) for d in dims])


# --------------------------------------------------------------------------
# P1: per-node projection  h = x @ Wcat   (Wcat = [W | Wa_src | Wa_dst])
# --------------------------------------------------------------------------
def _build_proj(COLS, HC):
    nc = bass.Bass()
    xT = nc.dram_tensor("xT", [P, NROWS], BF16, kind="ExternalInput")
    wcat = nc.dram_tensor("wcat", [P, COLS], BF16, kind="ExternalInput")
    hb = nc.dram_tensor("hb", [NROWS, HC], BF16, kind="ExternalOutput")
    att = nc.dram_tensor("att", [NROWS, COLS - HC], F32, kind="ExternalOutput")

    with ExitStack() as ctx:
        tc = ctx.enter_context(tile.TileContext(nc))
        pers = ctx.enter_context(tc.tile_pool(name="pers", bufs=1))
        xp = ctx.enter_context(tc.tile_pool(name="xp", bufs=3))
        sb = ctx.enter_context(tc.tile_pool(name="sb", bufs=3))
        ps = ctx.enter_context(tc.tile_pool(name="ps", bufs=4, space="PSUM"))

        wc = pers.tile([P, COLS], BF16)
        nc.sync.dma_start(out=wc[:], in_=wcat[:, :])
        for t in range(T):
            xt = xp.tile([P, P], BF16, tag="xt")
            nc.sync.dma_start(out=xt[:], in_=xT[:, t * P:(t + 1) * P])
            pg = ps.tile([P, COLS], F32, tag="pg")
            nc.tensor.matmul(out=pg[:], lhsT=xt[:], rhs=wc[:],
                             start=True, stop=True)
            ht = sb.tile([P, HC], BF16, tag="ht")
            nc.scalar.copy(out=ht[:], in_=pg[:, :HC])
            at = sb.tile([P, COLS - HC], F32, tag="at")
            nc.vector.tensor_scalar_add(out=at[:], in0=pg[:, HC:], scalar1=0.0)
            nc.sync.dma_start(out=hb[t * P:(t + 1) * P, :], in_=ht[:])
            nc.sync.dma_start(out=att[t * P:(t + 1) * P, :], in_=at[:])

    _split_waits(nc)
    return nc


# --------------------------------------------------------------------------
# P2/P3: edge aggregation over the slot grid (slot payload = gathered h)
# --------------------------------------------------------------------------
def _build_agg(KT, H, C, relu, proj_cols):
    """KT: per-tile slot counts.  Slot payload dim = HC = H*C (bf16).
    proj_cols: if nonzero, fuse out-row projection (relu'd) through w2c
    [P, proj_cols] and emit that instead of the raw aggregation."""
    HC = H * C
    SK = sum(KT)
    OUTC = proj_cols if proj_cols else HC

    nc = bass.Bass()
    hs = nc.dram_tensor("hs", [P, SK * HC], BF16, kind="ExternalInput")
    asr = nc.dram_tensor("asr", [P, SK * H], F32, kind="ExternalInput")
    ads = nc.dram_tensor("ads", [P, T * H], F32, kind="ExternalInput")
    warr = nc.dram_tensor("warr", [P, SK], F32, kind="ExternalInput")
    invc = nc.dram_tensor("invc", [P, T], F32, kind="ExternalInput")
    kk = nc.dram_tensor("kk", [P, H], F32, kind="ExternalInput")
    bvec = nc.dram_tensor("bvec", [P, HC], F32, kind="ExternalInput")
    if proj_cols:
        w2c = nc.dram_tensor("w2c", [P, proj_cols], BF16, kind="ExternalInput")
        idt = nc.dram_tensor("idt", [P, P], F32, kind="ExternalInput")
    outp = nc.dram_tensor("out", [NROWS, OUTC], F32, kind="ExternalOutput")

    with ExitStack() as ctx:
        tc = ctx.enter_context(tile.TileContext(nc))
        pers = ctx.enter_context(tc.tile_pool(name="pers", bufs=1))
        hp = ctx.enter_context(tc.tile_pool(name="hp", bufs=3))
        sb = ctx.enter_context(tc.tile_pool(name="sb", bufs=3))
        if proj_cols:
            ps = ctx.enter_context(tc.tile_pool(name="ps", bufs=3, space="PSUM"))

        asr_t = pers.tile([P, SK * H], F32)
        nc.sync.dma_start(out=asr_t[:], in_=asr[:, :])
        ads_t = pers.tile([P, T * H], F32)
        nc.sync.dma_start(out=ads_t[:], in_=ads[:, :])
        wall = pers.tile([P, SK], F32)
        nc.sync.dma_start(out=wall[:], in_=warr[:, :])
        iva = pers.tile([P, T], F32)
        nc.sync.dma_start(out=iva[:], in_=invc[:, :])
        kt = pers.tile([P, H], F32)
        nc.sync.dma_start(out=kt[:], in_=kk[:, :])
        bt = pers.tile([P, HC], F32)
        nc.sync.dma_start(out=bt[:], in_=bvec[:, :])
        if proj_cols:
            w2t = pers.tile([P, proj_cols], BF16)
            nc.sync.dma_start(out=w2t[:], in_=w2c[:, :])
            idtt = pers.tile([P, P], F32)
            nc.sync.dma_start(out=idtt[:], in_=idt[:, :])

        kpitch = kt[:].ap[0][0]
        cb = 0
        for t in range(T):
            K = KT[t]
            HS = hp.tile([P, K * HC], BF16, tag="HS")
            nc.sync.dma_start(out=HS[:], in_=hs[:, cb * HC:(cb + K) * HC])

            # alpha = a_src[slot] + a_dst[dst] + w*k_h           [P, K, H]
            A = sb.tile([P, K * H], F32, tag="A")
            a0 = A[:]
            apitch = a0.ap[0][0]
            A3 = _ap(a0, 0, [(apitch, P), (H, K), (1, H)])
            asrc_b = _ap(asr_t[:], cb * H,
                         [(asr_t[:].ap[0][0], P), (H, K), (1, H)])
            adst_b = _ap(ads_t[:], t * H,
                         [(ads_t[:].ap[0][0], P), (0, K), (1, H)])
            nc.vector.tensor_tensor(out=A3, in0=asrc_b, in1=adst_b,
                                    op=mybir.AluOpType.add)
            wt0 = wall[:, cb:cb + K]
            wpitch = wt0.ap[0][0]
            WK = sb.tile([P, K * H], F32, tag="WK")
            WK3 = _ap(WK[:], 0, [(WK[:].ap[0][0], P), (H, K), (1, H)])
            w_b = _ap(wt0, 0, [(wpitch, P), (1, K), (0, H)])
            kk_b = _ap(kt[:], 0, [(kpitch, P), (0, K), (1, H)])
            nc.gpsimd.tensor_tensor(out=WK3, in0=w_b, in1=kk_b,
                                    op=mybir.AluOpType.mult)
            nc.vector.tensor_tensor(out=A3, in0=A3, in1=WK3,
                                    op=mybir.AluOpType.add)
            # self-loop alpha correction at slot K-1: += k_h * (sum w)*invc
            LA = sb.tile([P, 1], F32, tag="LA")
            nc.vector.tensor_reduce(out=LA[:], in_=wt0,
                                    axis=mybir.AxisListType.X,
                                    op=mybir.AluOpType.add)
            nc.gpsimd.tensor_tensor(out=LA[:], in0=LA[:], in1=iva[:, t:t + 1],
                                    op=mybir.AluOpType.mult)
            A_self = _ap(a0, (K - 1) * H, [(apitch, P), (1, H)])
            kk_b2 = _ap(kt[:], 0, [(kpitch, P), (1, H)])
            nc.vector.scalar_tensor_tensor(out=A_self, in0=kk_b2,
                                           scalar=LA[:], in1=A_self,
                                           op0=mybir.AluOpType.mult,
                                           op1=mybir.AluOpType.add)
            # leaky relu in one STT, then exp (ACT) in bf16
            AL = sb.tile([P, K * H], F32, tag="AL")
            nc.vector.scalar_tensor_tensor(out=AL[:], in0=A[:],
                                           scalar=NEG_SLOPE, in1=A[:],
                                           op0=mybir.AluOpType.mult,
                                           op1=mybir.AluOpType.max)
            PP = sb.tile([P, K * H], BF16, tag="PP")
            nc.scalar.activation(out=PP[:], in_=AL[:],
                                 func=mybir.ActivationFunctionType.Exp)
            # PROD[p, k, h, c] = HS[p, k, hc] * PP[p, k, h]      (Pool)
            PROD = hp.tile([P, K * HC], BF16, tag="PROD")
            p0 = PROD[:]
            ppitch = p0.ap[0][0]
            h0 = HS[:]
            hpitch = h0.ap[0][0]
            pp0 = PP[:]
            pppitch = pp0.ap[0][0]
            PROD4 = _ap(p0, 0, [(ppitch, P), (HC, K), (C, H), (1, C)])
            HS4 = _ap(h0, 0, [(hpitch, P), (HC, K), (C, H), (1, C)])
            PPb = _ap(pp0, 0, [(pppitch, P), (H, K), (1, H), (0, C)])
            nc.gpsimd.tensor_tensor(out=PROD4, in0=HS4, in1=PPb,
                                    op=mybir.AluOpType.mult)
            # O[p, hc] = sum_k PROD ; Z[p, h] = sum_k PP        (DVE)
            O = sb.tile([P, HC], F32, tag="O")
            Or = _ap(O[:], 0, [(O[:].ap[0][0], P), (1, HC)])
            PRODr = _ap(p0, 0, [(ppitch, P), (1, HC), (HC, K)])
            nc.vector.tensor_reduce(out=Or, in_=PRODr,
                                    axis=mybir.AxisListType.X,
                                    op=mybir.AluOpType.add)
            Z = sb.tile([P, H], F32, tag="Z")
            PPr = _ap(pp0, 0, [(pppitch, P), (1, H), (H, K)])
            nc.vector.tensor_reduce(out=Z[:], in_=PPr,
                                    axis=mybir.AxisListType.X,
                                    op=mybir.AluOpType.add)
            Zr = sb.tile([P, H], F32, tag="Zr")
            nc.vector.reciprocal(out=Zr[:], in_=Z[:])
            # R = O * (1/Z) + b  (+ relu)
            R = sb.tile([P, HC], F32, tag="R")
            for h in range(H):
                nc.vector.scalar_tensor_tensor(
                    out=R[:, h * C:(h + 1) * C], in0=O[:, h * C:(h + 1) * C],
                    scalar=Zr[:, h:h + 1], in1=bt[:, h * C:(h + 1) * C],
                    op0=mybir.AluOpType.mult, op1=mybir.AluOpType.add)
            if relu:
                nc.vector.tensor_scalar_max(out=R[:], in0=R[:], scalar1=0.0)
            if proj_cols:
                # h2 rows for this tile: (R^T)^T @ w2c via PE transpose+mm
                tp = ps.tile([P, P], F32, tag="tp")
                nc.tensor.transpose(out=tp[:], in_=R[:], identity=idtt[:])
                rt = sb.tile([P, P], BF16, tag="rt")
                nc.scalar.copy(out=rt[:], in_=tp[:])
                h2p = ps.tile([P, proj_cols], F32, tag="h2p")
                nc.tensor.matmul(out=h2p[:], lhsT=rt[:], rhs=w2t[:],
                                 start=True, stop=True)
                h2s = sb.tile([P, proj_cols], F32, tag="h2s")
                nc.scalar.copy(out=h2s[:], in_=h2p[:])
                nc.sync.dma_start(out=outp[t * P:(t + 1) * P, :], in_=h2s[:])
            else:
                nc.sync.dma_start(out=outp[t * P:(t + 1) * P, :], in_=R[:])
            cb += K

    _split_waits(nc)
    return nc


# --------------------------------------------------------------------------
# host-side planning (identical partition to baseline)
# --------------------------------------------------------------------------
def _plan(edge_index):
    src = np.asarray(edge_index[0], dtype=np.int64)
    dst = np.asarray(edge_index[1], dtype=np.int64)
    deg = np.bincount(dst, minlength=N)
    order = np.argsort(-deg, kind="stable")          # rank -> node
    rank_of = np.empty(N, np.int64)
    rank_of[order] = np.arange(N)
    core_of = (rank_of % NCORES).astype(np.int64)
    loc_of = (rank_of // NCORES).astype(np.int64)

    KT = []
    for t in range(T):
        r0 = min(1024 * t, N - 1)
        KT.append(int(deg[order[r0]]) + 1)
    KT = [max(k, 1) for k in KT]
    cbs = np.concatenate([[0], np.cumsum(KT)])       # chunk bases

    # per-edge slot position: k = occurrence index within its dst
    eorder = np.argsort(dst, kind="stable")
    starts = np.concatenate([[0], np.cumsum(deg)])
    kpos_sorted = np.arange(E) - starts[dst[eorder]]
    kpos = np.empty(E, np.int64)
    kpos[eorder] = kpos_sorted

    e_core = core_of[dst]
    e_loc = loc_of[dst]
    e_t = e_loc >> 7
    e_p = e_loc & 127
    e_scol = cbs[e_t] + kpos                         # slot column (of SK)

    # self-slot coords per core: nodes order[c::8] in local order
    return dict(src=src, dst=dst, deg=deg, order=order, core_of=core_of,
                loc_of=loc_of, KT=KT, cbs=cbs, e_core=e_core, e_t=e_t,
                e_p=e_p, e_scol=e_scol)


def _gather_inputs(plan, hb_full, att_full, ew, kvec, bias, H, C, w2c=None):
    """Build per-core input maps for one aggregation layer.
    hb_full: [N, H*C] bf16 node payloads; att_full: [N, 2H] f32
    (cols [0:H]=a_src, [H:2H]=a_dst)."""
    HC = H * C
    KT, cbs = plan["KT"], plan["cbs"]
    SK = int(cbs[-1])
    src, e_core = plan["src"], plan["e_core"]
    e_scol, e_p = plan["e_scol"], plan["e_p"]
    order, deg = plan["order"], plan["deg"]
    KTa = np.array(KT)

    maps = []
    for c in range(NCORES):
        m = e_core == c
        hsr = np.zeros((P, SK, HC), NPBF)
        asr = np.full((P, SK, H), -1e4, np.float32)
        war = np.zeros((P, SK), np.float32)
        hsr[e_p[m], e_scol[m]] = hb_full[src[m]]
        asr[e_p[m], e_scol[m]] = att_full[src[m], :H]
        war[e_p[m], e_scol[m]] = ew[m]
        # self slots at k = K_t - 1
        nodes = order[c::NCORES]
        loc = np.arange(nodes.size)
        tt = loc >> 7
        pp = loc & 127
        self_col = cbs[tt] + KTa[tt] - 1
        hsr[pp, self_col] = hb_full[nodes]
        asr[pp, self_col] = att_full[nodes, :H]
        ads = np.zeros((P, T, H), np.float32)
        ads[pp, tt] = att_full[nodes, H:]
        iv = np.ones((P, T), np.float32)
        iv[pp, tt] = 1.0 / np.maximum(deg[nodes], 1.0)
        mp = {
            "hs": np.ascontiguousarray(hsr.reshape(P, SK * HC)),
            "asr": np.ascontiguousarray(asr.reshape(P, SK * H)),
            "ads": np.ascontiguousarray(ads.reshape(P, T * H)),
            "warr": war,
            "invc": iv,
            "kk": np.tile(kvec.reshape(1, H).astype(np.float32), (P, 1)),
            "bvec": np.tile(bias.reshape(1, -1).astype(np.float32), (P, 1)),
        }
        if w2c is not None:
            mp["w2c"] = w2c
            mp["idt"] = np.eye(P, dtype=np.float32)
        maps.append(mp)
    return maps


def _collect(plan, results, key):
    stack = np.stack([np.asarray(r[key]) for r in results])
    return stack[plan["core_of"], plan["loc_of"], :]


def _wcat(W, att_src, att_dst, H, C):
    """[128, H*C + 2H]: [W | W@A_src | W@A_dst]."""
    Wa_s = np.stack([W[:, h * C:(h + 1) * C] @ att_src[h] for h in range(H)], 1)
    Wa_d = np.stack([W[:, h * C:(h + 1) * C] @ att_dst[h] for h in range(H)], 1)
    return np.concatenate([W, Wa_s, Wa_d], axis=1).astype(np.float32)


def kernel(x, edge_index, edge_weight, W1, att_src1, att_dst1, W_edge1,
           att_edge1, b1, W2, att_src2, att_dst2, W_edge2, att_edge2, b2):
    global LAST_EXEC_NS
    LAST_EXEC_NS = []
    trace = os.environ.get("BASSGNN_TRACE", "0") == "1"

    x = np.asarray(x, np.float32)
    ew = np.asarray(edge_weight, np.float32).reshape(-1)
    plan = _plan(np.asarray(edge_index))
    core_ids = list(range(NCORES))

    k1 = np.array([W_edge1[0, h * 64:(h + 1) * 64] @ att_edge1[h]
                   for h in range(2)], np.float32)
    k2 = np.array([W_edge2[0, :64] @ att_edge2[0]], np.float32)
    W1c = _wcat(np.asarray(W1, np.float32), np.asarray(att_src1),
                np.asarray(att_dst1), 2, 64)
    W2c = _wcat(np.asarray(W2, np.float32), np.asarray(att_src2),
                np.asarray(att_dst2), 1, 64)

    # ---- P1: h1 = x @ W1c per node ----
    order = plan["order"]
    xT = np.ascontiguousarray(x.T).astype(NPBF)          # [128, N]
    nc1 = _build_proj(132, 128)
    maps1 = []
    for c in range(NCORES):
        nodes = order[c::NCORES]
        xTc = np.zeros((P, NROWS), NPBF)
        xTc[:, :nodes.size] = xT[:, nodes]
        maps1.append({"xT": xTc, "wcat": np.tile(W1c.astype(NPBF), (1, 1))})
    r1 = run_bass_kernel_spmd(nc1, maps1, core_ids, trace=trace)
    if trace:
        LAST_EXEC_NS.append(r1.exec_time_ns)
    h1b = _collect(plan, r1.results, "hb")               # [N, 128] bf16
    att1 = _collect(plan, r1.results, "att")             # [N, 4] f32

    # ---- P2: layer-1 aggregation + fused layer-2 projection ----
    nc2 = _build_agg(plan["KT"], 2, 64, relu=True, proj_cols=66)
    w2cb = W2c.astype(NPBF)
    maps2 = _gather_inputs(plan, h1b, att1, ew, k1, np.asarray(b1), 2, 64,
                           w2c=w2cb)
    r2 = run_bass_kernel_spmd(nc2, maps2, core_ids, trace=trace)
    if trace:
        LAST_EXEC_NS.append(r2.exec_time_ns)
    h2 = _collect(plan, r2.results, "out")               # [N, 66] f32
    h2b = h2[:, :64].astype(NPBF)
    att2 = h2[:, 64:66].astype(np.float32)               # [a_src2 | a_dst2]

    # ---- P3: layer-2 aggregation ----
    nc3 = _build_agg(plan["KT"], 1, 64, relu=False, proj_cols=0)
    maps3 = _gather_inputs(plan, h2b, att2, ew, k2, np.asarray(b2), 1, 64)
    r3 = run_bass_kernel_spmd(nc3, maps3, core_ids, trace=trace)
    if trace:
        LAST_EXEC_NS.append(r3.exec_time_ns)
    return _collect(plan, r3.results, "out").astype(np.float32)


# revision 14
# speedup vs baseline: 2.2047x; 1.5770x over previous
"""Two-layer GATConv (PyG-style, edge_dim=1, add_self_loops fill='mean') on
8 trn2 NeuronCores.

Strategy (v3: project-once, gather-h, contiguous-inner layouts)
---------------------------------------------------------------
Destinations are partitioned across the 8 cores (degree-sorted, dealt
round-robin).  Three device programs per kernel call:

  P1  per-node projection h1 = x @ [W1 | W1@A_src1 | W1@A_dst1]
      (bf16 matmul, one persistent xT load, grouped output DMAs).
  P2  layer-1 edge aggregation over a [128 dst x K_t slot] grid whose
      slot payloads are HOST-GATHERED h1 rows (bf16, hc-major k-inner
      blocks).  alpha on Pool, leaky+exp(+Z via accum_out) on ACT,
      normalized-attention product on DVE (all-bf16), bf16 fold-halving
      + f32 segment-reduce on DVE, bias on Pool, relu on ACT, fused
      layer-2 projection on PE.  Outputs h2 rows.
  P3  layer-2 edge aggregation, same grid, slots gathered from h2.

All FLOPs run on device; the host only moves data (gather/scatter of
rows, dtype rounding).
"""
import copy
import os

import numpy as np
import ml_dtypes

import concourse.bass as bass
import concourse.mybir as mybir
import concourse.tile as tile
from contextlib import ExitStack
from concourse.bass_utils import run_bass_kernel_spmd

NCORES = 8
P = 128
N = 50000
E = 800000
NPC = N // NCORES            # 6250 dsts per core
T = (NPC + P - 1) // P       # 49 tiles
NROWS = T * P                # 6272 rows per core (incl pad dsts)
G = 7                        # tiles per output-DMA group (49 = 7*7)
NEG_SLOPE = 0.2

F32 = mybir.dt.float32
BF16 = mybir.dt.bfloat16
NPBF = ml_dtypes.bfloat16

LAST_EXEC_NS = []


# --------------------------------------------------------------------------
# walrus workaround: cap sync waits per instruction (see v1 notes)
# --------------------------------------------------------------------------
def _split_waits(nc, limit=1):
    sem = nc.alloc_semaphore("wsplit_tmpl_sem")
    tmpl = {}
    for eng_ty, eng in nc.engines.items():
        tmpl[eng_ty] = eng.wait_ge(sem, 0).ins
    tmpl_names = {mi.name for mi in tmpl.values()}
    for f in nc.m.functions:
        for bb in f.blocks:
            insts = [i for i in bb.instructions if i.name not in tmpl_names]
            out = []
            for inst in insts:
                si = inst.sync_info
                waits = list(si.on_wait) if si and si.on_wait else []
                tn = type(inst).__name__
                eff = 0 if (tn == "InstDrain" or "Branch" in tn) else limit
                if len(waits) > eff:
                    head = waits[:-eff] if eff else waits
                    for w in head:
                        c = copy.deepcopy(tmpl[inst.engine])
                        c.name = f"I-wsplit-{nc.next_id()}"
                        c.sync_info = mybir.SyncInfo(on_wait=[w], on_update=[])
                        out.append(c)
                    inst.sync_info = mybir.SyncInfo(
                        on_wait=waits[-eff:] if eff else [],
                        on_update=list(si.on_update) if si.on_update else [],
                    )
                out.append(inst)
            bb.instructions = out


def _ap(root, extra_off, dims):
    return bass.AP(root.tensor, root.offset + extra_off, [list(d) for d in dims])


# --------------------------------------------------------------------------
# P1: per-node projection  h = x @ Wcat   (Wcat = [W | Wa_src | Wa_dst])
# --------------------------------------------------------------------------
def _build_proj(COLS, HC):
    AC = COLS - HC
    nc = bass.Bass()
    xT = nc.dram_tensor("xT", [P, NROWS], BF16, kind="ExternalInput")
    wcat = nc.dram_tensor("wcat", [P, COLS], BF16, kind="ExternalInput")
    hb = nc.dram_tensor("hb", [NROWS, HC], BF16, kind="ExternalOutput")
    att = nc.dram_tensor("att", [NROWS, AC], F32, kind="ExternalOutput")

    with ExitStack() as ctx:
        tc = ctx.enter_context(tile.TileContext(nc))
        pers = ctx.enter_context(tc.tile_pool(name="pers", bufs=1))
        sb = ctx.enter_context(tc.tile_pool(name="sb", bufs=2))
        ps = ctx.enter_context(tc.tile_pool(name="ps", bufs=4, space="PSUM"))

        wc = pers.tile([P, COLS], BF16)
        nc.sync.dma_start(out=wc[:], in_=wcat[:, :])
        xa = pers.tile([P, NROWS], BF16)
        nc.sync.dma_start(out=xa[:], in_=xT[:, :])

        hb_root = hb[:, :]
        att_root = att[:, :]
        for g in range(T // G):
            HG = sb.tile([P, G * HC], BF16, tag="HG")
            AG = sb.tile([P, G * AC], F32, tag="AG")
            for j in range(G):
                t = g * G + j
                pg = ps.tile([P, COLS], F32, tag="pg")
                nc.tensor.matmul(out=pg[:], lhsT=xa[:, t * P:(t + 1) * P],
                                 rhs=wc[:], start=True, stop=True)
                nc.scalar.copy(out=HG[:, j * HC:(j + 1) * HC], in_=pg[:, :HC])
                nc.vector.tensor_scalar_add(out=AG[:, j * AC:(j + 1) * AC],
                                            in0=pg[:, HC:], scalar1=0.0)
            hb_ap = _ap(hb_root, g * G * P * HC,
                        [(HC, P), (P * HC, G), (1, HC)])
            nc.sync.dma_start(out=hb_ap, in_=HG[:])
            att_ap = _ap(att_root, g * G * P * AC,
                         [(AC, P), (P * AC, G), (1, AC)])
            nc.sync.dma_start(out=att_ap, in_=AG[:])

    _split_waits(nc)
    return nc


# --------------------------------------------------------------------------
# P2/P3: edge aggregation over the slot grid (slot payload = gathered h)
# --------------------------------------------------------------------------
def _build_agg(KT, H, C, relu, proj_cols):
    """hs blocks per tile: [HC, K_t] (hc-major, k contiguous).
    asr: [P, H*SK] (h-major).  ads: [P, H*T]."""
    HC = H * C
    SK = sum(KT)
    OUTC = proj_cols if proj_cols else HC

    nc = bass.Bass()
    hs = nc.dram_tensor("hs", [P, SK * HC], BF16, kind="ExternalInput")
    asr = nc.dram_tensor("asr", [P, H * SK], F32, kind="ExternalInput")
    ads = nc.dram_tensor("ads", [P, H * T], F32, kind="ExternalInput")
    warr = nc.dram_tensor("warr", [P, SK], F32, kind="ExternalInput")
    invc = nc.dram_tensor("invc", [P, T], F32, kind="ExternalInput")
    kk = nc.dram_tensor("kk", [P, H], F32, kind="ExternalInput")
    bvec = nc.dram_tensor("bvec", [P, HC], F32, kind="ExternalInput")
    if proj_cols:
        w2c = nc.dram_tensor("w2c", [P, proj_cols], BF16, kind="ExternalInput")
        idt = nc.dram_tensor("idt", [P, P], BF16, kind="ExternalInput")
    outp = nc.dram_tensor("out", [NROWS, OUTC], F32, kind="ExternalOutput")

    with ExitStack() as ctx:
        tc = ctx.enter_context(tile.TileContext(nc))
        pers = ctx.enter_context(tc.tile_pool(name="pers", bufs=1))
        hp = ctx.enter_context(tc.tile_pool(name="hp", bufs=3))
        sb = ctx.enter_context(tc.tile_pool(name="sb", bufs=3))
        og = ctx.enter_context(tc.tile_pool(name="og", bufs=2))
        if proj_cols:
            ps = ctx.enter_context(tc.tile_pool(name="ps", bufs=3, space="PSUM"))

        asr_t = pers.tile([P, H * SK], F32)
        nc.sync.dma_start(out=asr_t[:], in_=asr[:, :])
        ads_t = pers.tile([P, H * T], F32)
        nc.sync.dma_start(out=ads_t[:], in_=ads[:, :])
        wall = pers.tile([P, SK], F32)
        nc.sync.dma_start(out=wall[:], in_=warr[:, :])
        iva = pers.tile([P, T], F32)
        nc.sync.dma_start(out=iva[:], in_=invc[:, :])
        kt = pers.tile([P, H], F32)
        nc.sync.dma_start(out=kt[:], in_=kk[:, :])
        bt = pers.tile([P, HC], F32)
        nc.sync.dma_start(out=bt[:], in_=bvec[:, :])
        if proj_cols:
            w2t = pers.tile([P, proj_cols], BF16)
            nc.sync.dma_start(out=w2t[:], in_=w2c[:, :])
            idtt = pers.tile([P, P], BF16)
            nc.sync.dma_start(out=idtt[:], in_=idt[:, :])

        kpitch = kt[:].ap[0][0]
        aspitch = asr_t[:].ap[0][0]
        adpitch = ads_t[:].ap[0][0]
        out_root = outp[:, :]
        cb = 0
        OG = None
        for t in range(T):
            K = KT[t]
            j = t % G
            if j == 0:
                OG = og.tile([P, G * OUTC], F32, tag="OG")
            HS = hp.tile([P, K * HC], BF16, tag="HS")
            nc.sync.dma_start(out=HS[:], in_=hs[:, cb * HC:(cb + K) * HC])

            # alpha[p, h, k] = a_src[slot] + a_dst[dst] + w*k_h   (Pool)
            A = sb.tile([P, H * K], F32, tag="A")
            a0 = A[:]
            apitch = a0.ap[0][0]
            A3 = _ap(a0, 0, [(apitch, P), (K, H), (1, K)])
            asrc_b = _ap(asr_t[:], cb, [(aspitch, P), (SK, H), (1, K)])
            adst_b = _ap(ads_t[:], t, [(adpitch, P), (T, H), (0, K)])
            nc.gpsimd.tensor_tensor(out=A3, in0=asrc_b, in1=adst_b,
                                    op=mybir.AluOpType.add)
            wt0 = wall[:, cb:cb + K]
            wpitch = wt0.ap[0][0]
            WK = sb.tile([P, H * K], F32, tag="WK")
            WK3 = _ap(WK[:], 0, [(WK[:].ap[0][0], P), (K, H), (1, K)])
            w_b = _ap(wt0, 0, [(wpitch, P), (0, H), (1, K)])
            kk_b = _ap(kt[:], 0, [(kpitch, P), (1, H), (0, K)])
            nc.gpsimd.tensor_tensor(out=WK3, in0=w_b, in1=kk_b,
                                    op=mybir.AluOpType.mult)
            nc.gpsimd.tensor_tensor(out=A3, in0=A3, in1=WK3,
                                    op=mybir.AluOpType.add)
            # self-loop alpha correction at k = K-1
            LA = sb.tile([P, 1], F32, tag="LA")
            nc.vector.tensor_reduce(out=LA[:], in_=wt0,
                                    axis=mybir.AxisListType.X,
                                    op=mybir.AluOpType.add)
            nc.gpsimd.tensor_tensor(out=LA[:], in0=LA[:], in1=iva[:, t:t + 1],
                                    op=mybir.AluOpType.mult)
            A_self = _ap(a0, K - 1, [(apitch, P), (K, H)])
            kk_b2 = _ap(kt[:], 0, [(kpitch, P), (1, H)])
            nc.vector.scalar_tensor_tensor(out=A_self, in0=kk_b2,
                                           scalar=LA[:], in1=A_self,
                                           op0=mybir.AluOpType.mult,
                                           op1=mybir.AluOpType.add)
            # leaky relu (DVE STT) + exp (ACT); Z via accum_out
            AL = sb.tile([P, H * K], F32, tag="AL")
            nc.vector.scalar_tensor_tensor(out=AL[:], in0=A[:],
                                           scalar=NEG_SLOPE, in1=A[:],
                                           op0=mybir.AluOpType.mult,
                                           op1=mybir.AluOpType.max)
            PP = sb.tile([P, H * K], BF16, tag="PP")
            Z = sb.tile([P, H], F32, tag="Z")
            for h in range(H):
                nc.scalar.activation(out=PP[:, h * K:(h + 1) * K],
                                     in_=AL[:, h * K:(h + 1) * K],
                                     func=mybir.ActivationFunctionType.Exp,
                                     accum_out=Z[:, h:h + 1])
            Zr = sb.tile([P, H], F32, tag="Zr")
            nc.vector.reciprocal(out=Zr[:], in_=Z[:])
            PPn = sb.tile([P, H * K], BF16, tag="PPn")
            pn0 = PPn[:]
            pnpitch = pn0.ap[0][0]
            Zr_b = _ap(Zr[:], 0, [(Zr[:].ap[0][0], P), (1, H), (0, K)])
            PP3 = _ap(PP[:], 0, [(PP[:].ap[0][0], P), (K, H), (1, K)])
            PPn3 = _ap(pn0, 0, [(pnpitch, P), (K, H), (1, K)])
            nc.vector.tensor_tensor(out=PPn3, in0=PP3, in1=Zr_b,
                                    op=mybir.AluOpType.mult)
            # PROD[p, h, c, k] = HS[p, h, c, k] * PPn[p, h, k]    (DVE bf16)
            PROD = hp.tile([P, HC * K], BF16, tag="PROD")
            p0 = PROD[:]
            ppitch = p0.ap[0][0]
            h0 = HS[:]
            hpitch = h0.ap[0][0]
            PROD4 = _ap(p0, 0, [(ppitch, P), (C * K, H), (K, C), (1, K)])
            HS4 = _ap(h0, 0, [(hpitch, P), (C * K, H), (K, C), (1, K)])
            PPn_b = _ap(pn0, 0, [(pnpitch, P), (K, H), (0, C), (1, K)])
            nc.vector.tensor_tensor(out=PROD4, in0=HS4, in1=PPn_b,
                                    op=mybir.AluOpType.mult)
            # one f32 fold (pairs k, k+K/2) split Pool/DVE, then f32 reduce
            nf = K // 2
            F1 = hp.tile([P, HC * nf], F32, tag="F1")
            f0 = F1[:]
            fpitch = f0.ap[0][0]
            SPLIT = 96 if H == 2 else 56    # hc rows handled by Pool
            for eng, lo, hi in ((nc.gpsimd, 0, SPLIT), (nc.vector, SPLIT, HC)):
                dstap = _ap(f0, lo * nf, [(fpitch, P), (nf, hi - lo), (1, nf)])
                s0ap = _ap(p0, lo * K, [(ppitch, P), (K, hi - lo), (1, nf)])
                s1ap = _ap(p0, lo * K + nf, [(ppitch, P), (K, hi - lo), (1, nf)])
                eng.tensor_tensor(out=dstap, in0=s0ap, in1=s1ap,
                                  op=mybir.AluOpType.add)
            O = sb.tile([P, HC], F32, tag="O")
            Or = _ap(O[:], 0, [(O[:].ap[0][0], P), (1, HC)])
            F1r = _ap(f0, 0, [(fpitch, P), (nf, HC), (1, nf)])
            nc.vector.tensor_reduce(out=Or, in_=F1r,
                                    axis=mybir.AxisListType.X,
                                    op=mybir.AluOpType.add)
            # + bias (Pool); b is zero in this workload but kept general
            if proj_cols:
                Ob = sb.tile([P, HC], F32, tag="Ob")
                nc.gpsimd.tensor_tensor(out=Ob[:], in0=O[:], in1=bt[:],
                                        op=mybir.AluOpType.add)
                R = sb.tile([P, HC], BF16, tag="R")
                nc.scalar.activation(out=R[:], in_=Ob[:],
                                     func=mybir.ActivationFunctionType.Relu)
                tp = ps.tile([P, P], BF16, tag="tp")
                nc.tensor.transpose(out=tp[:], in_=R[:], identity=idtt[:])
                rt = sb.tile([P, P], BF16, tag="rt")
                nc.scalar.copy(out=rt[:], in_=tp[:])
                h2p = ps.tile([P, proj_cols], F32, tag="h2p")
                nc.tensor.matmul(out=h2p[:], lhsT=rt[:], rhs=w2t[:],
                                 start=True, stop=True)
                nc.scalar.copy(out=OG[:, j * OUTC:(j + 1) * OUTC], in_=h2p[:])
            else:
                nc.gpsimd.tensor_tensor(out=OG[:, j * OUTC:(j + 1) * OUTC],
                                        in0=O[:], in1=bt[:],
                                        op=mybir.AluOpType.add)
            if j == G - 1:
                g0i = t - G + 1
                out_ap = _ap(out_root, g0i * P * OUTC,
                             [(OUTC, P), (P * OUTC, G), (1, OUTC)])
                nc.sync.dma_start(out=out_ap, in_=OG[:])
            cb += K

    _split_waits(nc)
    return nc


# --------------------------------------------------------------------------
# host-side planning (identical partition to baseline)
# --------------------------------------------------------------------------
def _plan(edge_index):
    src = np.asarray(edge_index[0], dtype=np.int64)
    dst = np.asarray(edge_index[1], dtype=np.int64)
    deg = np.bincount(dst, minlength=N)
    order = np.argsort(-deg, kind="stable")
    rank_of = np.empty(N, np.int64)
    rank_of[order] = np.arange(N)
    core_of = (rank_of % NCORES).astype(np.int64)
    loc_of = (rank_of // NCORES).astype(np.int64)

    KT = []
    for t in range(T):
        r0 = min(1024 * t, N - 1)
        k = int(deg[order[r0]]) + 1
        KT.append(k + (k & 1))            # even K -> clean fold pairing
    KT = [max(k, 2) for k in KT]
    cbs = np.concatenate([[0], np.cumsum(KT)])

    eorder = np.argsort(dst, kind="stable")
    starts = np.concatenate([[0], np.cumsum(deg)])
    kpos_sorted = np.arange(E) - starts[dst[eorder]]
    kpos = np.empty(E, np.int64)
    kpos[eorder] = kpos_sorted

    e_core = core_of[dst]
    e_loc = loc_of[dst]
    e_t = e_loc >> 7
    e_p = e_loc & 127
    e_scol = cbs[e_t] + kpos

    return dict(src=src, dst=dst, deg=deg, order=order, core_of=core_of,
                loc_of=loc_of, KT=KT, cbs=cbs, e_core=e_core, e_t=e_t,
                e_p=e_p, e_scol=e_scol, kpos=kpos)


def _gather_inputs(plan, hb_full, att_full, ew, kvec, bias, H, C, w2c=None):
    """Per-core input maps for one aggregation layer.
    hb_full: [N, H*C] bf16; att_full: [N, 2H] f32 ([a_src | a_dst])."""
    HC = H * C
    KT, cbs = plan["KT"], plan["cbs"]
    SK = int(cbs[-1])
    src, e_core = plan["src"], plan["e_core"]
    e_p, e_t, kpos = plan["e_p"], plan["e_t"], plan["kpos"]
    e_scol = plan["e_scol"]
    order, deg = plan["order"], plan["deg"]
    KTa = np.array(KT)

    maps = []
    for c in range(NCORES):
        m = e_core == c
        asr = np.full((P, H, SK), -1e4, np.float32)
        war = np.zeros((P, SK), np.float32)
        asr[e_p[m], :, e_scol[m]] = att_full[src[m], :H]
        war[e_p[m], e_scol[m]] = ew[m]
        nodes = order[c::NCORES]
        loc = np.arange(nodes.size)
        tt = loc >> 7
        pp = loc & 127
        self_col = cbs[tt] + KTa[tt] - 1
        asr[pp, :, self_col] = att_full[nodes, :H]
        ads = np.zeros((P, H, T), np.float32)
        ads[pp, :, tt] = att_full[nodes, H:]
        iv = np.ones((P, T), np.float32)
        iv[pp, tt] = 1.0 / np.maximum(deg[nodes], 1.0)
        # hs: per-tile [P, HC, K_t] blocks (hc-major, k inner)
        hsr = np.zeros((P, SK * HC), NPBF)
        ep_m, et_m, kp_m, src_m = e_p[m], e_t[m], kpos[m], src[m]
        for t in range(T):
            K = KTa[t]
            blk = np.zeros((P, HC, K), NPBF)
            sel = et_m == t
            blk[ep_m[sel], :, kp_m[sel]] = hb_full[src_m[sel]]
            tn = tt == t
            blk[pp[tn], :, K - 1] = hb_full[nodes[tn]]
            hsr[:, cbs[t] * HC:(cbs[t] + K) * HC] = blk.reshape(P, HC * K)
        mp = {
            "hs": hsr,
            "asr": np.ascontiguousarray(asr.reshape(P, H * SK)),
            "ads": np.ascontiguousarray(ads.reshape(P, H * T)),
            "warr": war,
            "invc": iv,
            "kk": np.tile(kvec.reshape(1, H).astype(np.float32), (P, 1)),
            "bvec": np.tile(bias.reshape(1, -1).astype(np.float32), (P, 1)),
        }
        if w2c is not None:
            mp["w2c"] = w2c
            mp["idt"] = np.eye(P, dtype=NPBF)
        maps.append(mp)
    return maps


def _collect(plan, results, key):
    stack = np.stack([np.asarray(r[key]) for r in results])
    return stack[plan["core_of"], plan["loc_of"], :]


def _wcat(W, att_src, att_dst, H, C):
    Wa_s = np.stack([W[:, h * C:(h + 1) * C] @ att_src[h] for h in range(H)], 1)
    Wa_d = np.stack([W[:, h * C:(h + 1) * C] @ att_dst[h] for h in range(H)], 1)
    return np.concatenate([W, Wa_s, Wa_d], axis=1).astype(np.float32)


def kernel(x, edge_index, edge_weight, W1, att_src1, att_dst1, W_edge1,
           att_edge1, b1, W2, att_src2, att_dst2, W_edge2, att_edge2, b2):
    global LAST_EXEC_NS
    LAST_EXEC_NS = []
    trace = os.environ.get("BASSGNN_TRACE", "0") == "1"

    x = np.asarray(x, np.float32)
    ew = np.asarray(edge_weight, np.float32).reshape(-1)
    plan = _plan(np.asarray(edge_index))
    core_ids = list(range(NCORES))

    k1 = np.array([W_edge1[0, h * 64:(h + 1) * 64] @ att_edge1[h]
                   for h in range(2)], np.float32)
    k2 = np.array([W_edge2[0, :64] @ att_edge2[0]], np.float32)
    W1c = _wcat(np.asarray(W1, np.float32), np.asarray(att_src1),
                np.asarray(att_dst1), 2, 64)
    W2c = _wcat(np.asarray(W2, np.float32), np.asarray(att_src2),
                np.asarray(att_dst2), 1, 64)

    # ---- P1 ----
    order = plan["order"]
    xT = np.ascontiguousarray(x.T).astype(NPBF)
    nc1 = _build_proj(132, 128)
    maps1 = []
    for c in range(NCORES):
        nodes = order[c::NCORES]
        xTc = np.zeros((P, NROWS), NPBF)
        xTc[:, :nodes.size] = xT[:, nodes]
        maps1.append({"xT": xTc, "wcat": W1c.astype(NPBF)})
    r1 = run_bass_kernel_spmd(nc1, maps1, core_ids, trace=trace)
    if trace:
        LAST_EXEC_NS.append(r1.exec_time_ns)
    h1b = _collect(plan, r1.results, "hb")
    att1 = _collect(plan, r1.results, "att")

    # ---- P2 ----
    nc2 = _build_agg(plan["KT"], 2, 64, relu=True, proj_cols=66)
    maps2 = _gather_inputs(plan, h1b, att1, ew, k1, np.asarray(b1), 2, 64,
                           w2c=W2c.astype(NPBF))
    r2 = run_bass_kernel_spmd(nc2, maps2, core_ids, trace=trace)
    if trace:
        LAST_EXEC_NS.append(r2.exec_time_ns)
    h2 = _collect(plan, r2.results, "out")
    h2b = h2[:, :64].astype(NPBF)
    att2 = h2[:, 64:66].astype(np.float32)

    # ---- P3 ----
    nc3 = _build_agg(plan["KT"], 1, 64, relu=False, proj_cols=0)
    maps3 = _gather_inputs(plan, h2b, att2, ew, k2, np.asarray(b2), 1, 64)
    r3 = run_bass_kernel_spmd(nc3, maps3, core_ids, trace=trace)
    if trace:
        LAST_EXEC_NS.append(r3.exec_time_ns)
    return _collect(plan, r3.results, "out").astype(np.float32)
